# revision 85
# baseline (speedup 1.0000x reference)
"""Trainium2 Bass kernel for nn_DoubleNet (two GATNet branches + avg-pool + linear).

Strategy (8 NeuronCores):
  - Cores 0-3 run branch A, cores 4-7 run branch B (same SPMD program,
    different input data per core).
  - Within a branch, dst nodes are sharded contiguously across the 4 cores.
  - Per GAT layer:
      dense phase (sharded): each core computes z = x @ [W | W@al | W@ar] for
        its OWN 5000-node shard only and stages it as fp8 rows of 512B:
          [z0|1|z1|1|z2|1] fp8 cols 0..386, then per-node attention factors
          Pl=e^el Ql=e^.2el Pr=e^er Qr=e^.2er as bf16 at bytes 400..423
        (exp(leaky_relu(el+er)) == max(Pl*Pr, Ql*Qr), so the edge phase needs
        no Act exp on its critical chain).  The 4 shards are AllGathered into
        the full 20000-row table; a local er-pair table (row 2d+m =
        [Pr,Qr](d) | [Pr,Qr](d+m)) is built by tiny strided DMAs.
      edge phase: edges are pre-sorted by dst (host side) and processed in
        256-edge chunks: pair (2i, 2i+1) shares partition i as slots 0/1.
        Self-loops make consecutive sorted dsts differ by <=1, so one
        512B er-pair row per pair (straddle bit m baked into the host index)
        replaces a per-edge er gather: one 1024-desc z-gather per 4-chunk
        unit plus one 1024-desc er gather per TWO units.  wt = max(Pl*Pr,
        Ql*Qr) is 3 DVE ALU ops per unit.  Per (chunk, slot, head) a
        wt-scaled one-hot (DVE, built narrowly into a pre-zeroed [S, S+W)
        window class) drives a bf16 x fp8 matmul scatter-adding messages AND
        the denominator into PSUM per 128-dst block.
      block epilogue: agg rows are normalized on Act (Copy with 1/den scale),
        transposed on PE (bf16), x_next^T = Wl^T @ agg^T + bias written to
        the xT SBUF slice on Act (Identity + AP bias), keeping the dense
        chain off the congested DVE queue.
  - Final layer pools via a host-precomputed gid one-hot matmul; host sums the
    per-core partial pools and applies the output linear.
"""

import sys

sys.path.insert(0, "/opt/trn_rl_repo")

import numpy as np


# ---------------------------------------------------------------------------
# configuration
# ---------------------------------------------------------------------------

class Cfg:
    def __init__(self, N=20000, G=128, H=3, EMB=128, F=128, n_cores=8,
                 neg_slope=0.2):
        assert F == 128 and EMB == 128 and H == 3
        self.N, self.G, self.H, self.EMB, self.F = N, G, H, EMB, F
        self.n_cores = n_cores
        self.gpb = n_cores // 2            # cores per branch
        assert N % self.gpb == 0
        self.SH = N // self.gpb            # dst nodes per core
        self.NB = -(-self.SH // 128)       # dst blocks per core
        self.SHP = self.NB * 128           # padded shard size
        self.neg_slope = neg_slope
        self.ZC = 512                      # fp8 table row: 512 bytes
        self.GC = 4                        # 256-edge chunks per z-gather
        self.er_w = 2                      # units per er-pair gather
        self.dma_scratch = 16384           # SWDGE ring (>16K hangs HW)
        self.actheavy = False
        self.piecewise = True
        self.poolcopy = False
        self.interleave = True
        self.PR = 512                      # AllGather piece rows
        self.NQ = 1                        # SWDGE queues (round-robin gathers)
        self.narrow = True                 # windowed one-hot builds
        self.wq = 8                        # window quantization granularity
        self.lookahead = 4                 # gather prefetch distance (units)
        self.psb_bufs = 3                  # PSUM block accumulators
        self.psz_bufs = 2                  # PSUM dense-phase buffers
        self.lh_bufs = 12                  # one-hot buffers per window class
        self.g_bufs = 8                    # gather tile buffers (units deep)
        self.wt_la = 3                     # wt-chain lookahead (units)
        self.prezero_psb = False           # Act-prezero psb; all-narrow builds
        self.ablate_wtdep = False          # measurement: builds skip wt
        self.wt_on_pool = False            # wt muls on gpsimd vs DVE
        self.ablate_exp = False            # measurement: skip Act exp


# ---------------------------------------------------------------------------
# host-side data prep
# ---------------------------------------------------------------------------

def _prep_edges(cfg, src, dst, q):
    """Edges of one core (dst in its shard), dst-sorted, fake rows added."""
    lo = q * cfg.SH
    sel = (dst >= lo) & (dst < lo + cfg.SH)
    es = src[sel].astype(np.int64)
    ed = (dst[sel].astype(np.int64) - lo)
    nfake = cfg.NB * 128 - cfg.SH
    if nfake:
        es = np.concatenate([es, np.zeros(nfake, np.int64)])
        ed = np.concatenate([ed, np.arange(cfg.SH, cfg.NB * 128, dtype=np.int64)])
    order = np.argsort(ed, kind="stable")
    es, ed = es[order], ed[order]
    cnt = np.bincount(ed // 128, minlength=cfg.NB)
    return es, ed, cnt, lo


def _rowmap(cfg, n):
    """Global node id -> padded piece-major zaug row (see do_zgather)."""
    n = np.asarray(n, np.int64)
    PR, SH, gpb = cfg.PR, cfg.SH, cfg.gpb
    q, local = n // SH, n % SH
    p = local // PR
    return gpb * PR * p + q * PR + (local - p * PR)


def _chunk_windows(cfg, per_core_edges, nc_b):
    """Per-subchunk one-hot build windows [S, S+W) shared by all cores.

    Chunks hold 256 dst-sorted edges as two slots (even/odd edge of each
    pair) of 128 partitions; a subchunk is (chunk, slot), id 2*cc+s.  The
    matmul stays full-width (PE tile_position limits PSUM partition
    offsets); only the DVE build is narrowed, writing lh[:, S:S+W] of a
    [128,128] tile whose margins stay zero.  Classes are quantized to
    (S16, W=32), (S32, W=64) or (0, 128) to bound the tile-tag count."""
    NB = cfg.NB
    cum = np.concatenate([[0], np.cumsum(nc_b)]).astype(int)
    NSUB = 2 * int(cum[-1])
    S_arr = np.full(NSUB, 128, np.int64)
    E_arr = np.zeros(NSUB, np.int64)
    for es, ed, lo in per_core_edges:
        epos = np.searchsorted(ed, np.arange(0, NB * 128 + 1, 128))
        for b in range(NB):
            s, e = epos[b], epos[b + 1]
            rel = ed[s:e] - b * 128  # ascending within the block
            n = e - s
            for ci in range(-(-n // 256)):
                i0, i1 = ci * 256, min((ci + 1) * 256, n)
                for sl in (0, 1):
                    r = rel[i0 + sl:i1:2]
                    if len(r) == 0:
                        continue
                    sub = 2 * (cum[b] + ci) + sl
                    S_arr[sub] = min(S_arr[sub], r[0])
                    E_arr[sub] = max(E_arr[sub], r[-1] + 1)
    S_cls = np.zeros(NSUB, np.int64)
    W_cls = np.full(NSUB, 128, np.int64)
    if cfg.narrow:
        for cc in range(NSUB):
            if E_arr[cc] == 0:
                S_cls[cc], W_cls[cc] = 0, 32
                continue
            s16 = min((S_arr[cc] // 16) * 16, 96)
            s32 = min((S_arr[cc] // 32) * 32, 64)
            if E_arr[cc] - s16 <= 32:
                S_cls[cc], W_cls[cc] = s16, 32
            elif E_arr[cc] - s32 <= 64:
                S_cls[cc], W_cls[cc] = s32, 64
    # first/last subchunk of each block stay full width so the PSUM
    # start/stop matmuls cover all 128 dst rows (unless psb is pre-zeroed)
    if not cfg.prezero_psb:
        for b in range(NB):
            for sub in (2 * cum[b], 2 * cum[b + 1] - 1):
                S_cls[sub], W_cls[sub] = 0, 128
    return S_cls, W_cls


def _pack_core(cfg, es, ed, lo, nc_b, S_cls):
    """Build flat (block, 256-chunk, slot, pair) arrays for one core.

    Edges stay dst-sorted; chunk c's edges 2i / 2i+1 share partition i as
    slots 0/1 (z-gather groups 2c and 2c+1).  Because every dst has a
    self-loop, consecutive sorted dsts differ by at most 1, so one er-pair
    row [er(d), er(d+1)] at d = dst(edge 2i) serves both slots; mask says
    whether slot1 uses er(d+1)."""
    TOT = int(nc_b.sum())
    zsrc = np.zeros(TOT * 256, np.int16)
    eidx = np.zeros(TOT * 128, np.int16)
    dst3 = np.full(TOT * 256, -1.0, np.float32)
    epos = np.searchsorted(ed, np.arange(0, cfg.NB * 128 + 1, 128))
    cum = np.concatenate([[0], np.cumsum(nc_b)]).astype(int)
    for b in range(cfg.NB):
        s, e = epos[b], epos[b + 1]
        n = e - s
        rel = (ed[s:e] - b * 128).astype(np.float32)
        for ci in range(-(-n // 256)):
            cc = cum[b] + ci
            i0, i1 = ci * 256, min((ci + 1) * 256, n)
            for sl in (0, 1):
                idx = np.arange(i0 + sl, i1, 2)
                o = (2 * cc + sl) * 128
                zsrc[o:o + len(idx)] = _rowmap(cfg, es[s + idx])
                dst3[o:o + len(idx)] = rel[idx] - S_cls[2 * cc + sl]
            ev = np.arange(i0, i1, 2)
            od = np.arange(i0 + 1, i1, 2)
            o = cc * 128
            # erTab2 row 2d+m = [er(d), er(d+m)]; m = pair straddles dsts
            m = np.zeros(len(ev), np.int64)
            m[:len(od)] = ed[s + od] > ed[s + ev[:len(od)]]
            eidx[o:o + len(ev)] = 2 * ed[s + ev] + m
    # index tiles: flat i -> (partition i%16, col i//16), replicated to 128
    def wrap(a):
        return np.tile(a.reshape(-1, 16).T, (8, 1)).copy()
    d3 = dst3.reshape(2 * TOT, 128).T.copy()
    return wrap(zsrc), wrap(eidx), d3


def _prep_branch_weights(cfg, W1, al1, ar1, b1, Wn, aln, arn, bn, Wl, bl):
    H, EMB = cfg.H, cfg.EMB

    def waug(W, al, ar):
        K = W.shape[0]
        out = np.zeros((K, 390), np.float32)
        out[:, :384] = W
        for h in range(H):
            out[:, 384 + h] = W[:, h * EMB:(h + 1) * EMB] @ al[h]
            out[:, 387 + h] = W[:, h * EMB:(h + 1) * EMB] @ ar[h]
        return out

    wl3 = Wl.reshape(3, 128, EMB).astype(np.float32)
    blp1 = (b1 @ Wl + bl).astype(np.float32)
    blpn = (bn @ Wl + bl).astype(np.float32)
    return waug(W1, al1, ar1), waug(Wn, aln, arn), wl3, blp1, blpn


# ---------------------------------------------------------------------------
# device program
# ---------------------------------------------------------------------------

def build_program(cfg, nc_b, S_cls=None, W_cls=None, timing_mode=False):
    import concourse.bass as bass
    import concourse.mybir as mybir
    import concourse.tile as tile
    from concourse import bacc

    dt = mybir.dt
    f32 = dt.float32
    bf16 = dt.bfloat16
    fp8 = dt.float8e4
    Alu = mybir.AluOpType
    Act = mybir.ActivationFunctionType

    NB, SH, GC, ZC = cfg.NB, cfg.SH, cfg.GC, cfg.ZC
    SHP = cfg.SHP
    TOT = int(nc_b.sum())
    cum = np.concatenate([[0], np.cumsum(nc_b)]).astype(int)
    if S_cls is None:
        S_cls = np.zeros(2 * TOT, np.int64)
        W_cls = np.full(2 * TOT, 128, np.int64)
    gpb = cfg.gpb
    groups = [list(range(gpb)), list(range(gpb, 2 * gpb))]
    blk_of = np.repeat(np.arange(NB), nc_b)
    n_units = -(-TOT // GC)
    ends_in_unit = {}
    for b in range(NB):
        ends_in_unit.setdefault((cum[b + 1] - 1) // GC, []).append(b)

    nc = bacc.Bacc("TRN2", target_bir_lowering=False, debug=False,
                   num_devices=cfg.n_cores,
                   num_swdge_queues=cfg.NQ,
                   dynamic_dma_scratch_size=cfg.dma_scratch)

    # inputs -----------------------------------------------------------------
    xT0 = nc.dram_tensor("xT0", [128, SHP], bf16, kind="ExternalInput")
    waug1_d = nc.dram_tensor("waug1", [128, 390], bf16, kind="ExternalInput")
    waugn_d = nc.dram_tensor("waugn", [128, 390], bf16, kind="ExternalInput")
    wl3_d = nc.dram_tensor("wl3", [3, 128, 128], bf16, kind="ExternalInput")
    blp1_d = nc.dram_tensor("blp1", [128, 1], f32, kind="ExternalInput")
    blpn_d = nc.dram_tensor("blpn", [128, 1], f32, kind="ExternalInput")
    iota_d = nc.dram_tensor("iota", [128, 128], bf16, kind="ExternalInput")
    ident_d = nc.dram_tensor("ident", [128, 128], bf16, kind="ExternalInput")
    dst3_d = nc.dram_tensor("dst3", [128, 2 * TOT], f32, kind="ExternalInput")
    zidx_d = nc.dram_tensor("zidx", [128, TOT * 16], dt.int16,
                            kind="ExternalInput")
    eidx_d = nc.dram_tensor("eidx", [128, TOT * 8], dt.int16, kind="ExternalInput")
    poolw_d = nc.dram_tensor("poolw", [NB, 128, 128], bf16, kind="ExternalInput")
    pool_out = nc.dram_tensor("pool_out", [128, 128], f32, kind="ExternalOutput")

    # internal DRAM (double buffered across layers) ---------------------------
    zshs = [nc.dram_tensor(f"zsh{i}", [SHP, ZC], fp8) for i in range(2)]
    zaugs = [nc.dram_tensor(f"zaug{i}", [cfg.gpb * cfg.SHP, ZC], fp8)
             for i in range(2)]
    # er-pair rows: row 2d+m = [er(d) | er(d+m) | pad] (m = straddle bit,
    # host-baked into the pair index), 512B stride so the pair gather
    # dodges the sub-512B DMA latency penalty and needs no on-chip select
    erTabs = [nc.dram_tensor(f"ertab{i}", [2 * SHP, ZC], fp8)
              for i in range(2)]

    # zaug rows are PIECE-MAJOR over PADDED pieces: piece p (PR shard-rows,
    # PR % 512 == 0) holds the 4 cores' sub-shards contiguously, so each
    # piecewise AllGather output is a dense linear block, reshaped to
    # 128-partition views (HW collectives expect [128, X]-shaped operands).
    PR = cfg.PR
    assert PR % 512 == 0 and SHP % PR == 0
    NPC = SHP // PR
    AA = PR // 128

    def do_zgather(nc, par, p):
        """AllGather zsh piece p into every core's zaug (piece-major)."""
        zsh, zaug = zshs[par], zaugs[par]
        base = gpb * PR * p
        if timing_mode:
            for j in range(gpb):
                nc.sync.dma_start(
                    zaug.ap()[base + j * PR:base + (j + 1) * PR, :],
                    zsh.ap()[p * PR:(p + 1) * PR, :])
        else:
            nc.gpsimd.collective_compute(
                "AllGather", mybir.AluOpType.bypass, replica_groups=groups,
                ins=[zsh.ap()[p * PR:(p + 1) * PR, :].bitcast(bf16)
                     .rearrange("(p a) z -> p (a z)", p=128, a=AA)],
                outs=[zaug.ap()[base:base + gpb * PR, :].bitcast(bf16)
                      .rearrange("(q p a) z -> q p (a z)", q=gpb, p=128,
                                 a=AA)])

    with tile.TileContext(nc) as tc:
        cpool = tc.alloc_tile_pool(name="const", bufs=1)
        # persistent SBUF state
        xTs = [cpool.tile([128, 512], bf16, tag=f"xT{i}", name=f"xT{i}")
               for i in range(NB // 4)]
        waug1 = cpool.tile([128, 390], bf16, tag="waug1")
        waugn = cpool.tile([128, 390], bf16, tag="waugn")
        wl3 = cpool.tile([128, 3, 128], bf16, tag="wl3")
        blp1 = cpool.tile([128, 1], f32, tag="blp1")
        iota = cpool.tile([128, 128], bf16, tag="iota")
        ident = cpool.tile([128, 128], bf16, tag="ident")
        dst3 = cpool.tile([128, 2 * TOT], f32, tag="dst3")
        zidx = cpool.tile([128, TOT * 16], dt.int16, tag="zidx")
        eidx = cpool.tile([128, TOT * 8], dt.int16, tag="eidx")
        poolw = cpool.tile([128, NB, 128], bf16, tag="poolw")
        zt4s = [cpool.tile([128, 4, ZC], fp8, tag=f"zt4_{i}", name=f"zt4_{i}")
                for i in range(3)]
        zpsb = cpool.tile([128, 387], f32, tag="zpsb")
        nc.vector.memset(zpsb[:], 0.0)

        for i in range(NB // 4):
            nc.sync.dma_start(xTs[i][:], xT0.ap()[:, i * 512:(i + 1) * 512])
        nc.sync.dma_start(waug1[:], waug1_d.ap())
        nc.sync.dma_start(waugn[:], waugn_d.ap())
        nc.sync.dma_start(wl3[:], wl3_d.ap().rearrange("k p m -> p k m"))
        nc.sync.dma_start(blp1[:], blp1_d.ap())
        nc.sync.dma_start(iota[:], iota_d.ap())
        nc.sync.dma_start(ident[:], ident_d.ap())
        nc.sync.dma_start(dst3[:], dst3_d.ap())
        nc.sync.dma_start(zidx[:], zidx_d.ap())
        nc.sync.dma_start(eidx[:], eidx_d.ap())
        nc.sync.dma_start(poolw[:], poolw_d.ap().rearrange("b p m -> p b m"))
        for zt4 in zt4s:
            # ones columns for the denominator; zero the padding tail once
            nc.vector.memset(zt4[:, :, 128:387:129], 1.0)
            nc.vector.memset(zt4[:, :, 387:512], 0.0)

        # defensively zero the consumed bytes of the er-pair tables so a
        # pair row read before its dense-phase write can never inject NaNs
        zf = cpool.tile([128, 2 * SHP // 128, 24], dt.uint8, tag="zf")
        nc.vector.memset(zf[:], 0)
        for et in erTabs:
            nc.sync.dma_start(
                et.ap().bitcast(dt.uint8)[:, 0:24].rearrange(
                    "(p a) c -> p a c", p=128), zf[:])

        psz_pool = tc.alloc_tile_pool(name="psz", bufs=cfg.psz_bufs,
                                      space="PSUM")
        g_pool = tc.alloc_tile_pool(name="g", bufs=cfg.g_bufs)
        r_pool = tc.alloc_tile_pool(name="r",
                                    bufs=cfg.g_bufs // cfg.er_w + 2)
        w_pool = tc.alloc_tile_pool(name="w", bufs=cfg.g_bufs)
        l_pool = tc.alloc_tile_pool(name="l", bufs=96)
        psb_pool = tc.alloc_tile_pool(name="psb", bufs=cfg.psb_bufs,
                                      space="PSUM")
        pst_pool = tc.alloc_tile_pool(name="pst", bufs=1, space="PSUM")
        psx_pool = tc.alloc_tile_pool(name="psx", bufs=1, space="PSUM")
        s_pool = tc.alloc_tile_pool(name="s", bufs=4)
        a_pool = tc.alloc_tile_pool(name="a", bufs=4)
        at_pool = tc.alloc_tile_pool(name="at", bufs=4)
        x_pool = tc.alloc_tile_pool(name="x", bufs=4)
        pp_pool = tc.alloc_tile_pool(name="pp", bufs=1, space="PSUM")

        ps_pool_acc = pp_pool.tile([128, 128], f32, tag="poolacc")

        def emit_dense_group(dlayer, tg):
            """Dense tiles 4tg..4tg+3 of layer `dlayer` -> fp8 z table shard."""
            par = dlayer % 2
            wa = waug1 if dlayer == 0 else waugn
            zt4 = zt4s[tg % 3]
            zt4b = zt4.bitcast(bf16)
            for j in range(4):
                t = tg * 4 + j
                psz = psz_pool.tile([128, 390], f32, tag="psz")
                nc.tensor.matmul(
                    psz[:], xTs[t // 4][:, (t % 4) * 128:(t % 4 + 1) * 128],
                    wa[:], start=True, stop=True)
                # GPSIMD cannot read PSUM on HW: one strided copy for the
                # three z segments (DVE/Act alternating) + per-node factors
                # Pl=e^el Ql=e^.2el Pr=e^er Qr=e^.2er (exp(lrelu(el+er)) ==
                # max(Pl*Pr, Ql*Qr)), so the edge phase needs no Act exp
                zs = zt4[:, j, 0:387].rearrange("p (s m) -> p s m", s=3, m=129)[
                    :, :, 0:128]
                zin = psz[:, 0:384].rearrange("p (s m) -> p s m", s=3, m=128)
                nc.scalar.copy(zs, zin)
                # [Pl|Ql|Pr|Qr] in two strided exps, fused by scale
                v = zt4b[:, j, 200:212].rearrange("p (q x) -> p q x", q=2)
                src2 = psz[:, 384:390].rearrange("p (q h) -> p q h", q=2)
                nc.scalar.activation(v[:, :, 0:3], src2, Act.Exp, scale=1.0)
                nc.scalar.activation(v[:, :, 3:6], src2, Act.Exp,
                                     scale=cfg.neg_slope)
            nc.sync.dma_start(
                zshs[par].ap()[tg * 512:(tg + 1) * 512, :]
                .rearrange("(c p) z -> p c z", c=4, p=128), zt4[:])
            if ((tg + 1) * 512) % PR == 0:
                do_zgather(nc, par, ((tg + 1) * 512) // PR - 1)
            # er-pair table rows for the previous 512-node group (whose +1
            # boundary row is now available); the very last group clamps its
            # final straddle row to itself (row SHP-1 is padding anyway).
            # half-layer batches keep the SP queue's DMACopy count low (each
            # dma_start holds the SP sequencer ~3us) while the first half is
            # still ready before the next layer's first er gathers need it
            def emit_ertab(d0, d1, last):
                et, zsh = erTabs[par], zshs[par]
                n = d1 - d0
                etv = et.ap()[2 * d0:2 * d1, :].rearrange(
                    "(d m) z -> d m z", m=2)
                src = zsh.ap()[d0:d1, 412:424]            # [Pr(d)|Qr(d)]
                nc.sync.dma_start(etv[:, 0, 0:12], src)
                nc.sync.dma_start(etv[:, 1, 0:12], src)
                nc.sync.dma_start(etv[:, 0, 12:24], src)
                n1 = n - 1 if last else n
                nc.sync.dma_start(etv[0:n1, 1, 12:24],
                                  zsh.ap()[d0 + 1:d0 + 1 + n1, 412:424])
                if last:
                    nc.sync.dma_start(etv[n - 1:n, 1, 12:24],
                                      zsh.ap()[SHP - 1:SHP, 412:424])
            half = (NB // 4) // 2
            if tg == half - 1:
                emit_ertab(0, 512 * half - 1, last=False)
            if tg == NB // 4 - 1:
                emit_ertab(512 * half - 1, SHP, last=True)

        # layer 0's dense phase runs standalone; layers 1-2 are emitted from
        # inside the previous layer's block ends (block b feeds dense tile b).
        for tg in range(NB // 4):
            emit_dense_group(0, tg)

        lh_uses = {}
        for layer in range(3):
            if not cfg.interleave and layer > 0:
                for tg in range(NB // 4):
                    emit_dense_group(layer, tg)
            zaug = zaugs[layer % 2]
            erTab = erTabs[layer % 2]
            # ---------------- edge phase (software pipelined) ---------------
            # unit = GC 256-edge chunks (2*GC z-gather groups, 1024 idxs);
            # er-pair gathers cover ERW units each (GC*ERW*128 idxs).
            ERW = cfg.er_w
            Gts, Rws, wts, psbs = {}, {}, {}, {}

            def emit_gathers(u):
                lo = u * GC
                gsz = min(GC, TOT - lo)
                Gt = g_pool.tile([128, 2 * GC, ZC], fp8, tag="G")
                nc.gpsimd.dma_gather(
                    Gt[:, 0:2 * gsz, :], zaug.ap(),
                    zidx[:, 16 * lo: 16 * (lo + gsz)],
                    num_idxs=gsz * 256, num_idxs_reg=gsz * 256,
                    elem_size=ZC, elem_step=ZC,
                    queue_num=(2 * u) % cfg.NQ)
                Gts[u] = (Gt, gsz)
                if u % ERW == 0:
                    c1 = min(lo + GC * ERW, TOT)
                    R = r_pool.tile([128, GC * ERW, ZC], fp8, tag="R")
                    nc.gpsimd.dma_gather(
                        R[:, 0:c1 - lo, :], erTab.ap(),
                        eidx[:, 8 * lo: 8 * c1],
                        num_idxs=(c1 - lo) * 128,
                        num_idxs_reg=(c1 - lo) * 128,
                        elem_size=ZC, elem_step=ZC,
                        queue_num=(2 * u + 1) % cfg.NQ)
                    Rws[u // ERW] = R

            def emit_wt(u):
                Gt, gsz = Gts[u]
                R = Rws[u // ERW]
                go = (u % ERW) * GC
                Gtb = Gt.bitcast(bf16)
                Rb = R.bitcast(bf16)
                # wt = max(Pl*Pr, Ql*Qr) == exp(leaky_relu(el+er)): pure DVE
                # ALU, no Act hop on the per-unit critical chain.  Pair rows
                # hold [Pr Qr] per slot as strided bf16 views.
                wt = w_pool.tile([128, 2 * GC, 3], f32, tag="wt")
                wtq = w_pool.tile([128, 2 * GC, 3], f32, tag="wtq")
                w4 = wt[:].rearrange("p (c s) h -> p c s h", s=2)[:, 0:gsz]
                q4 = wtq[:].rearrange("p (c s) h -> p c s h", s=2)[:, 0:gsz]
                rv = Rb[:, go:go + gsz, 0:12].rearrange(
                    "p c (s q h) -> p c s q h", s=2, q=2)
                gv = Gtb[:, 0:2 * gsz, 200:212].rearrange(
                    "p g (q h) -> p g q h", q=4)
                g4 = gv.rearrange("p (c s) q h -> p c s q h", s=2)
                eng = nc.gpsimd if cfg.wt_on_pool else nc.vector
                eng.tensor_tensor(
                    w4, g4[:, :, :, 0, :], rv[:, :, :, 0, :], Alu.mult)
                eng.tensor_tensor(
                    q4, g4[:, :, :, 1, :], rv[:, :, :, 1, :], Alu.mult)
                eng.tensor_tensor(
                    wt[:, 0:2 * gsz, :], wt[:, 0:2 * gsz, :],
                    wtq[:, 0:2 * gsz, :], Alu.max)
                wts[u] = wt

            def emit_compute(u):
                lo = u * GC
                hi = min(lo + GC, TOT)
                Gt, _ = Gts[u]
                wt = wts[u]
                for cc in range(lo, hi):
                    b = int(blk_of[cc])
                    if cc == cum[b]:
                        psbs[b] = psb_pool.tile([128, 387], f32, tag="psb",
                                                name="psb")
                        if cfg.prezero_psb:
                            nc.scalar.copy(psbs[b][:], zpsb[:])
                    psb = psbs[b]
                    c = cc - lo
                    for sl in (0, 1):
                        sub = 2 * cc + sl
                        S, W = int(S_cls[sub]), int(W_cls[sub])
                        tag = f"lh{S}_{W}"
                        for h in range(3):
                            lh = l_pool.tile([128, 128], bf16, tag=tag,
                                             bufs=cfg.lh_bufs, name="lh")
                            n_used = lh_uses[tag] = lh_uses.get(tag, 0) + 1
                            if W < 128 and n_used <= cfg.lh_bufs:
                                # fresh rotating buffer: zero the margins
                                # once; later builds rewrite exactly [S, S+W)
                                nc.vector.memset(lh[:], 0.0)
                            wsc = (dst3[:, sub:sub + 1] if cfg.ablate_wtdep
                                   else wt[:, 2 * c + sl, h:h + 1].opt())
                            nc.vector.tensor_scalar(
                                lh[:, S:S + W], iota[:, 0:W],
                                dst3[:, sub:sub + 1], wsc,
                                Alu.is_equal, Alu.mult)
                            nc.tensor.matmul(
                                psb[:, 129 * h:129 * h + 129], lh[:],
                                Gt[:, 2 * c + sl, 129 * h:129 * h + 129].opt(),
                                start=(not cfg.prezero_psb
                                       and sub == 2 * cum[b] and h == 0),
                                stop=(sub == 2 * cum[b + 1] - 1 and h == 2))

            def emit_block_end(b):
                psb = psbs.pop(b)
                r3 = s_pool.tile([128, 3], f32, tag="r3")
                # +eps: a denominator can only be 0 for padding rows (or a
                # lost race on the er tables); keep 1/den finite either way
                nc.vector.tensor_scalar_add(r3[:], psb[:, 128:387:129], 1e-12)
                nc.vector.reciprocal(r3[:], r3[:])
                agg = a_pool.tile([128, 384], bf16, tag="agg")
                for h in range(3):
                    nc.scalar.activation(
                        agg[:, 128 * h:128 * (h + 1)],
                        psb[:, 129 * h:129 * h + 128], Act.Copy,
                        scale=r3[:, h:h + 1])
                aggT = at_pool.tile([128, 3, 128], bf16, tag="aggT")
                pst = pst_pool.tile([128, 3, 128], bf16, tag="pst")
                for k in range(3):
                    nc.tensor.transpose(pst[:, k, :].opt(),
                                        agg[:, 128 * k:128 * (k + 1)],
                                        ident[:])
                nc.scalar.copy(aggT[:], pst[:])
                bw = min(128, SH - b * 128)
                if layer < 2:
                    psx = psx_pool.tile([128, 128], f32, tag="psx")
                    for k in range(3):
                        nc.tensor.matmul(psx[:], wl3[:, k, :].opt(),
                                         aggT[:, k, :].opt(),
                                         start=(k == 0), stop=(k == 2))
                    # bias-add and write x^T straight into the xT SBUF slice
                    # (Act engine; Identity accepts an AP bias unlike Copy)
                    xdst = xTs[b // 4][:, (b % 4) * 128:(b % 4) * 128 + bw]
                    nc.scalar.activation(xdst, psx[:, 0:bw], Act.Identity,
                                         bias=blp1[:])
                else:
                    psx = psx_pool.tile([128, 128], f32, tag="psx")
                    for k in range(3):
                        nc.tensor.matmul(psx[:], aggT[:, k, :].opt(),
                                         wl3[:, k, :].opt(),
                                         start=(k == 0), stop=(k == 2))
                    x3 = x_pool.tile([128, 128], bf16, tag="x3")
                    nc.scalar.copy(x3[:], psx[:])
                    nc.tensor.matmul(ps_pool_acc[:], poolw[:, b, :].opt(),
                                     x3[:], start=(b == 0), stop=(b == NB - 1))
                if cfg.interleave and layer < 2 and b % 4 == 3:
                    emit_dense_group(layer + 1, b // 4)

            LA = cfg.lookahead
            WL = cfg.wt_la
            for u0 in range(min(LA, n_units)):
                emit_gathers(u0)
            for u0 in range(min(WL, LA, n_units)):
                emit_wt(u0)
            for u in range(n_units):
                emit_compute(u)
                if u + LA < n_units:
                    emit_gathers(u + LA)
                # wt chains run several units ahead of compute so the Act
                # exp is never on the per-unit critical loop; their R
                # gathers landed LA-WL units earlier, so the DVE add never
                # head-of-line-blocks ready one-hot builds.
                if u + WL < n_units:
                    emit_wt(u + WL)
                if u >= 1:
                    for b in ends_in_unit.get(u - 1, []):
                        emit_block_end(b)
            for b in ends_in_unit.get(n_units - 1, []):
                emit_block_end(b)

            if layer == 0:
                nc.sync.dma_start(blp1[:], blpn_d.ap())

        po = x_pool.tile([128, 128], f32, tag="po")
        nc.vector.tensor_copy(po[:], ps_pool_acc[:])
        nc.sync.dma_start(pool_out.ap(), po[:])

        for p in (pp_pool, x_pool, at_pool, a_pool, s_pool,
                  psx_pool, pst_pool, psb_pool, l_pool, w_pool, r_pool,
                  g_pool, psz_pool, cpool):
            p.release()

    nc.compile()
    return nc


# ---------------------------------------------------------------------------
# top-level kernel
# ---------------------------------------------------------------------------

def _prepare(cfg, inputs):
    """Returns (nc_b, S_cls, W_cls, in_maps, host_meta)."""
    import ml_dtypes
    bf = ml_dtypes.bfloat16
    npf = np.asarray
    per_core_edges = []
    nc_b = np.zeros(cfg.NB, np.int64)
    for br, (s, d) in enumerate((("srcA", "dstA"), ("srcB", "dstB"))):
        src = npf(inputs[s]).astype(np.int64)
        dst = npf(inputs[d]).astype(np.int64)
        for q in range(cfg.gpb):
            es, ed, cnt, lo = _prep_edges(cfg, src, dst, q)
            per_core_edges.append((es, ed, lo))
            nc_b = np.maximum(nc_b, -(-cnt // 256))
    S_cls, W_cls = _chunk_windows(cfg, per_core_edges, nc_b)
    in_maps = []
    host_meta = {}
    iota = np.tile(np.arange(128, dtype=bf), (128, 1))
    ident = np.eye(128, dtype=bf)
    for br in range(2):
        sfx = "AB"[br]
        W1 = npf(inputs["W1" + sfx]); al1 = npf(inputs["al1" + sfx])
        ar1 = npf(inputs["ar1" + sfx]); b1 = npf(inputs["b1" + sfx])
        Wn = npf(inputs["Wn" + sfx]); aln = npf(inputs["aln" + sfx])
        arn = npf(inputs["arn" + sfx]); bn = npf(inputs["bn" + sfx])
        Wl = npf(inputs["Wl" + sfx]); bl = npf(inputs["bl" + sfx])
        gid = npf(inputs["gid" + sfx]).astype(np.int64)
        feats = npf(inputs["feats" + sfx]).astype(np.float32)
        waug1, waugn, wl3, blp1, blpn = _prep_branch_weights(
            cfg, W1, al1, ar1, b1, Wn, aln, arn, bn, Wl, bl)
        host_meta[sfx] = dict(blpn=blpn, gid=gid)
        for q in range(cfg.gpb):
            es, ed, lo = per_core_edges[br * cfg.gpb + q]
            zidx, eidx, dst3 = _pack_core(cfg, es, ed, lo, nc_b, S_cls)
            poolw = np.zeros((cfg.NB, 128, 128), bf)
            for b in range(cfg.NB):
                for i in range(min(128, cfg.SH - b * 128)):
                    n = lo + b * 128 + i
                    if n < cfg.N:
                        poolw[b, i, gid[n]] = 1.0
            xT0 = np.zeros((128, cfg.SHP), np.float32)
            xT0[:, :cfg.SH] = feats.T[:, lo:lo + cfg.SH]
            in_maps.append({
                "xT0": xT0.astype(bf), "waug1": waug1.astype(bf),
                "waugn": waugn.astype(bf),
                "wl3": wl3.astype(bf), "blp1": blp1.reshape(128, 1),
                "blpn": blpn.reshape(128, 1),
                "iota": iota, "ident": ident,
                "dst3": dst3, "zidx": zidx, "eidx": eidx, "poolw": poolw,
            })
    return nc_b, S_cls, W_cls, in_maps, host_meta


def _finalize(cfg, inputs, host_meta, pool_outs):
    """pool_outs: list of 8 [128,128] arrays -> full output [G,1] float64."""
    out = {}
    for br in range(2):
        sfx = "AB"[br]
        total = np.zeros((128, 128), np.float64)
        for q in range(cfg.gpb):
            total += pool_outs[br * cfg.gpb + q].astype(np.float64)
        gid = host_meta[sfx]["gid"]
        cnt = np.bincount(gid, minlength=128).astype(np.float64)
        total += cnt[:, None] * host_meta[sfx]["blpn"].astype(np.float64)[None, :]
        out[sfx] = (total / np.maximum(cnt[:, None], 1.0))[:cfg.G]
    cat = np.concatenate([out["A"], out["B"]], axis=1)
    Wo = np.asarray(inputs["Wo"]).astype(np.float64)
    bo = np.asarray(inputs["bo"]).astype(np.float64)
    return (cat @ Wo + bo).astype(np.float64)


_CACHE = {}


def kernel(**inputs):
    cfg = Cfg(N=inputs["featsA"].shape[0], G=128)
    nc_b, S_cls, W_cls, in_maps, host_meta = _prepare(cfg, inputs)
    key = ("prog", tuple(nc_b.tolist()), tuple(S_cls.tolist()),
           tuple(W_cls.tolist()))
    if key not in _CACHE:
        _CACHE[key] = build_program(cfg, nc_b, S_cls, W_cls)
    nc = _CACHE[key]
    from concourse.bass_utils import run_bass_kernel_spmd
    res = run_bass_kernel_spmd(nc, in_maps, list(range(cfg.n_cores)))
    pool_outs = [r["pool_out"] for r in res.results]
    return _finalize(cfg, inputs, host_meta, pool_outs)



# revision 88
# speedup vs baseline: 1.0009x; 1.0009x over previous
"""Trainium2 Bass kernel for nn_DoubleNet (two GATNet branches + avg-pool + linear).

Strategy (8 NeuronCores):
  - Cores 0-3 run branch A, cores 4-7 run branch B (same SPMD program,
    different input data per core).
  - Within a branch, dst nodes are sharded contiguously across the 4 cores.
  - Per GAT layer:
      dense phase (sharded): each core computes z = x @ [W | W@al | W@ar] for
        its OWN 5000-node shard only and stages it as fp8 rows of 512B:
          [z0|1|z1|1|z2|1] fp8 cols 0..386, then per-node attention factors
          Pl=e^el Ql=e^.2el Pr=e^er Qr=e^.2er as bf16 at bytes 400..423
        (exp(leaky_relu(el+er)) == max(Pl*Pr, Ql*Qr), so the edge phase needs
        no Act exp on its critical chain).  The 4 shards are AllGathered into
        the full 20000-row table; a local er-pair table (row 2d+m =
        [Pr,Qr](d) | [Pr,Qr](d+m)) is built by tiny strided DMAs.
      edge phase: edges are pre-sorted by dst (host side) and processed in
        256-edge chunks: pair (2i, 2i+1) shares partition i as slots 0/1.
        Self-loops make consecutive sorted dsts differ by <=1, so one
        512B er-pair row per pair (straddle bit m baked into the host index)
        replaces a per-edge er gather: one 1024-desc z-gather per 4-chunk
        unit plus one 1024-desc er gather per TWO units.  wt = max(Pl*Pr,
        Ql*Qr) is 3 DVE ALU ops per unit.  Per (chunk, slot, head) a
        wt-scaled one-hot (DVE, built narrowly into a pre-zeroed [S, S+W)
        window class) drives a bf16 x fp8 matmul scatter-adding messages AND
        the denominator into PSUM per 128-dst block.
      block epilogue: agg rows are normalized on Act (Copy with 1/den scale),
        transposed on PE (bf16), x_next^T = Wl^T @ agg^T + bias written to
        the xT SBUF slice on Act (Identity + AP bias), keeping the dense
        chain off the congested DVE queue.
  - Final layer pools via a host-precomputed gid one-hot matmul; host sums the
    per-core partial pools and applies the output linear.
"""

import sys

sys.path.insert(0, "/opt/trn_rl_repo")

import numpy as np


# ---------------------------------------------------------------------------
# configuration
# ---------------------------------------------------------------------------

class Cfg:
    def __init__(self, N=20000, G=128, H=3, EMB=128, F=128, n_cores=8,
                 neg_slope=0.2):
        assert F == 128 and EMB == 128 and H == 3
        self.N, self.G, self.H, self.EMB, self.F = N, G, H, EMB, F
        self.n_cores = n_cores
        self.gpb = n_cores // 2            # cores per branch
        assert N % self.gpb == 0
        self.SH = N // self.gpb            # dst nodes per core
        self.NB = -(-self.SH // 128)       # dst blocks per core
        self.SHP = self.NB * 128           # padded shard size
        self.neg_slope = neg_slope
        self.ZC = 512                      # fp8 table row: 512 bytes
        self.GC = 4                        # 256-edge chunks per z-gather
        self.er_w = 2                      # units per er-pair gather
        self.dma_scratch = 16384           # SWDGE ring (>16K hangs HW)
        self.actheavy = False
        self.piecewise = True
        self.poolcopy = False
        self.interleave = True
        self.PR = 512                      # AllGather piece rows
        self.NQ = 1                        # SWDGE queues (round-robin gathers)
        self.narrow = True                 # windowed one-hot builds
        self.wq = 8                        # window quantization granularity
        self.lookahead = 4                 # gather prefetch distance (units)
        self.psb_bufs = 2                  # PSUM block accumulators
        self.psz_bufs = 3                  # PSUM dense-phase buffers
        self.lh_bufs = 12                  # one-hot buffers per window class
        self.g_bufs = 8                    # gather tile buffers (units deep)
        self.wt_la = 3                     # wt-chain lookahead (units)
        self.prezero_psb = False           # Act-prezero psb; all-narrow builds
        self.ablate_wtdep = False          # measurement: builds skip wt
        self.wt_on_pool = False            # wt muls on gpsimd vs DVE
        self.ablate_exp = False            # measurement: skip Act exp


# ---------------------------------------------------------------------------
# host-side data prep
# ---------------------------------------------------------------------------

def _prep_edges(cfg, src, dst, q):
    """Edges of one core (dst in its shard), dst-sorted, fake rows added."""
    lo = q * cfg.SH
    sel = (dst >= lo) & (dst < lo + cfg.SH)
    es = src[sel].astype(np.int64)
    ed = (dst[sel].astype(np.int64) - lo)
    nfake = cfg.NB * 128 - cfg.SH
    if nfake:
        es = np.concatenate([es, np.zeros(nfake, np.int64)])
        ed = np.concatenate([ed, np.arange(cfg.SH, cfg.NB * 128, dtype=np.int64)])
    order = np.argsort(ed, kind="stable")
    es, ed = es[order], ed[order]
    cnt = np.bincount(ed // 128, minlength=cfg.NB)
    return es, ed, cnt, lo


def _rowmap(cfg, n):
    """Global node id -> padded piece-major zaug row (see do_zgather)."""
    n = np.asarray(n, np.int64)
    PR, SH, gpb = cfg.PR, cfg.SH, cfg.gpb
    q, local = n // SH, n % SH
    p = local // PR
    return gpb * PR * p + q * PR + (local - p * PR)


def _chunk_windows(cfg, per_core_edges, nc_b):
    """Per-subchunk one-hot build windows [S, S+W) shared by all cores.

    Chunks hold 256 dst-sorted edges as two slots (even/odd edge of each
    pair) of 128 partitions; a subchunk is (chunk, slot), id 2*cc+s.  The
    matmul stays full-width (PE tile_position limits PSUM partition
    offsets); only the DVE build is narrowed, writing lh[:, S:S+W] of a
    [128,128] tile whose margins stay zero.  Classes are quantized to
    (S16, W=32), (S32, W=64) or (0, 128) to bound the tile-tag count."""
    NB = cfg.NB
    cum = np.concatenate([[0], np.cumsum(nc_b)]).astype(int)
    NSUB = 2 * int(cum[-1])
    S_arr = np.full(NSUB, 128, np.int64)
    E_arr = np.zeros(NSUB, np.int64)
    for es, ed, lo in per_core_edges:
        epos = np.searchsorted(ed, np.arange(0, NB * 128 + 1, 128))
        for b in range(NB):
            s, e = epos[b], epos[b + 1]
            rel = ed[s:e] - b * 128  # ascending within the block
            n = e - s
            for ci in range(-(-n // 256)):
                i0, i1 = ci * 256, min((ci + 1) * 256, n)
                for sl in (0, 1):
                    r = rel[i0 + sl:i1:2]
                    if len(r) == 0:
                        continue
                    sub = 2 * (cum[b] + ci) + sl
                    S_arr[sub] = min(S_arr[sub], r[0])
                    E_arr[sub] = max(E_arr[sub], r[-1] + 1)
    S_cls = np.zeros(NSUB, np.int64)
    W_cls = np.full(NSUB, 128, np.int64)
    if cfg.narrow:
        for cc in range(NSUB):
            if E_arr[cc] == 0:
                S_cls[cc], W_cls[cc] = 0, 32
                continue
            s16 = min((S_arr[cc] // 16) * 16, 96)
            s32 = min((S_arr[cc] // 32) * 32, 64)
            if E_arr[cc] - s16 <= 32:
                S_cls[cc], W_cls[cc] = s16, 32
            elif E_arr[cc] - s32 <= 64:
                S_cls[cc], W_cls[cc] = s32, 64
    # first/last subchunk of each block stay full width so the PSUM
    # start/stop matmuls cover all 128 dst rows (unless psb is pre-zeroed)
    if not cfg.prezero_psb:
        for b in range(NB):
            for sub in (2 * cum[b], 2 * cum[b + 1] - 1):
                S_cls[sub], W_cls[sub] = 0, 128
    return S_cls, W_cls


def _pack_core(cfg, es, ed, lo, nc_b, S_cls):
    """Build flat (block, 256-chunk, slot, pair) arrays for one core.

    Edges stay dst-sorted; chunk c's edges 2i / 2i+1 share partition i as
    slots 0/1 (z-gather groups 2c and 2c+1).  Because every dst has a
    self-loop, consecutive sorted dsts differ by at most 1, so one er-pair
    row [er(d), er(d+1)] at d = dst(edge 2i) serves both slots; mask says
    whether slot1 uses er(d+1)."""
    TOT = int(nc_b.sum())
    zsrc = np.zeros(TOT * 256, np.int16)
    eidx = np.zeros(TOT * 128, np.int16)
    dst3 = np.full(TOT * 256, -1.0, np.float32)
    epos = np.searchsorted(ed, np.arange(0, cfg.NB * 128 + 1, 128))
    cum = np.concatenate([[0], np.cumsum(nc_b)]).astype(int)
    for b in range(cfg.NB):
        s, e = epos[b], epos[b + 1]
        n = e - s
        rel = (ed[s:e] - b * 128).astype(np.float32)
        for ci in range(-(-n // 256)):
            cc = cum[b] + ci
            i0, i1 = ci * 256, min((ci + 1) * 256, n)
            for sl in (0, 1):
                idx = np.arange(i0 + sl, i1, 2)
                o = (2 * cc + sl) * 128
                zsrc[o:o + len(idx)] = _rowmap(cfg, es[s + idx])
                dst3[o:o + len(idx)] = rel[idx] - S_cls[2 * cc + sl]
            ev = np.arange(i0, i1, 2)
            od = np.arange(i0 + 1, i1, 2)
            o = cc * 128
            # erTab2 row 2d+m = [er(d), er(d+m)]; m = pair straddles dsts
            m = np.zeros(len(ev), np.int64)
            m[:len(od)] = ed[s + od] > ed[s + ev[:len(od)]]
            eidx[o:o + len(ev)] = 2 * ed[s + ev] + m
    # index tiles: flat i -> (partition i%16, col i//16), replicated to 128
    def wrap(a):
        return np.tile(a.reshape(-1, 16).T, (8, 1)).copy()
    d3 = dst3.reshape(2 * TOT, 128).T.copy()
    return wrap(zsrc), wrap(eidx), d3


def _prep_branch_weights(cfg, W1, al1, ar1, b1, Wn, aln, arn, bn, Wl, bl):
    H, EMB = cfg.H, cfg.EMB

    def waug(W, al, ar):
        K = W.shape[0]
        out = np.zeros((K, 390), np.float32)
        out[:, :384] = W
        for h in range(H):
            out[:, 384 + h] = W[:, h * EMB:(h + 1) * EMB] @ al[h]
            out[:, 387 + h] = W[:, h * EMB:(h + 1) * EMB] @ ar[h]
        return out

    wl3 = Wl.reshape(3, 128, EMB).astype(np.float32)
    blp1 = (b1 @ Wl + bl).astype(np.float32)
    blpn = (bn @ Wl + bl).astype(np.float32)
    return waug(W1, al1, ar1), waug(Wn, aln, arn), wl3, blp1, blpn


# ---------------------------------------------------------------------------
# device program
# ---------------------------------------------------------------------------

def build_program(cfg, nc_b, S_cls=None, W_cls=None, timing_mode=False):
    import concourse.bass as bass
    import concourse.mybir as mybir
    import concourse.tile as tile
    from concourse import bacc

    dt = mybir.dt
    f32 = dt.float32
    bf16 = dt.bfloat16
    fp8 = dt.float8e4
    Alu = mybir.AluOpType
    Act = mybir.ActivationFunctionType

    NB, SH, GC, ZC = cfg.NB, cfg.SH, cfg.GC, cfg.ZC
    SHP = cfg.SHP
    TOT = int(nc_b.sum())
    cum = np.concatenate([[0], np.cumsum(nc_b)]).astype(int)
    if S_cls is None:
        S_cls = np.zeros(2 * TOT, np.int64)
        W_cls = np.full(2 * TOT, 128, np.int64)
    gpb = cfg.gpb
    groups = [list(range(gpb)), list(range(gpb, 2 * gpb))]
    blk_of = np.repeat(np.arange(NB), nc_b)
    n_units = -(-TOT // GC)
    ends_in_unit = {}
    for b in range(NB):
        ends_in_unit.setdefault((cum[b + 1] - 1) // GC, []).append(b)

    nc = bacc.Bacc("TRN2", target_bir_lowering=False, debug=False,
                   num_devices=cfg.n_cores,
                   num_swdge_queues=cfg.NQ,
                   dynamic_dma_scratch_size=cfg.dma_scratch)

    # inputs -----------------------------------------------------------------
    xT0 = nc.dram_tensor("xT0", [128, SHP], bf16, kind="ExternalInput")
    waug1_d = nc.dram_tensor("waug1", [128, 390], bf16, kind="ExternalInput")
    waugn_d = nc.dram_tensor("waugn", [128, 390], bf16, kind="ExternalInput")
    wl3_d = nc.dram_tensor("wl3", [3, 128, 128], bf16, kind="ExternalInput")
    blp1_d = nc.dram_tensor("blp1", [128, 1], f32, kind="ExternalInput")
    blpn_d = nc.dram_tensor("blpn", [128, 1], f32, kind="ExternalInput")
    iota_d = nc.dram_tensor("iota", [128, 128], bf16, kind="ExternalInput")
    ident_d = nc.dram_tensor("ident", [128, 128], bf16, kind="ExternalInput")
    dst3_d = nc.dram_tensor("dst3", [128, 2 * TOT], f32, kind="ExternalInput")
    zidx_d = nc.dram_tensor("zidx", [128, TOT * 16], dt.int16,
                            kind="ExternalInput")
    eidx_d = nc.dram_tensor("eidx", [128, TOT * 8], dt.int16, kind="ExternalInput")
    poolw_d = nc.dram_tensor("poolw", [NB, 128, 128], bf16, kind="ExternalInput")
    pool_out = nc.dram_tensor("pool_out", [128, 128], f32, kind="ExternalOutput")

    # internal DRAM (double buffered across layers) ---------------------------
    zshs = [nc.dram_tensor(f"zsh{i}", [SHP, ZC], fp8) for i in range(2)]
    zaugs = [nc.dram_tensor(f"zaug{i}", [cfg.gpb * cfg.SHP, ZC], fp8)
             for i in range(2)]
    # er-pair rows: row 2d+m = [er(d) | er(d+m) | pad] (m = straddle bit,
    # host-baked into the pair index), 512B stride so the pair gather
    # dodges the sub-512B DMA latency penalty and needs no on-chip select
    erTabs = [nc.dram_tensor(f"ertab{i}", [2 * SHP, ZC], fp8)
              for i in range(2)]

    # zaug rows are PIECE-MAJOR over PADDED pieces: piece p (PR shard-rows,
    # PR % 512 == 0) holds the 4 cores' sub-shards contiguously, so each
    # piecewise AllGather output is a dense linear block, reshaped to
    # 128-partition views (HW collectives expect [128, X]-shaped operands).
    PR = cfg.PR
    assert PR % 512 == 0 and SHP % PR == 0
    NPC = SHP // PR
    AA = PR // 128

    def do_zgather(nc, par, p):
        """AllGather zsh piece p into every core's zaug (piece-major)."""
        zsh, zaug = zshs[par], zaugs[par]
        base = gpb * PR * p
        if timing_mode:
            for j in range(gpb):
                nc.sync.dma_start(
                    zaug.ap()[base + j * PR:base + (j + 1) * PR, :],
                    zsh.ap()[p * PR:(p + 1) * PR, :])
        else:
            nc.gpsimd.collective_compute(
                "AllGather", mybir.AluOpType.bypass, replica_groups=groups,
                ins=[zsh.ap()[p * PR:(p + 1) * PR, :].bitcast(bf16)
                     .rearrange("(p a) z -> p (a z)", p=128, a=AA)],
                outs=[zaug.ap()[base:base + gpb * PR, :].bitcast(bf16)
                      .rearrange("(q p a) z -> q p (a z)", q=gpb, p=128,
                                 a=AA)])

    with tile.TileContext(nc) as tc:
        cpool = tc.alloc_tile_pool(name="const", bufs=1)
        # persistent SBUF state
        xTs = [cpool.tile([128, 512], bf16, tag=f"xT{i}", name=f"xT{i}")
               for i in range(NB // 4)]
        waug1 = cpool.tile([128, 390], bf16, tag="waug1")
        waugn = cpool.tile([128, 390], bf16, tag="waugn")
        wl3 = cpool.tile([128, 3, 128], bf16, tag="wl3")
        blp1 = cpool.tile([128, 1], f32, tag="blp1")
        iota = cpool.tile([128, 128], bf16, tag="iota")
        ident = cpool.tile([128, 128], bf16, tag="ident")
        dst3 = cpool.tile([128, 2 * TOT], f32, tag="dst3")
        zidx = cpool.tile([128, TOT * 16], dt.int16, tag="zidx")
        eidx = cpool.tile([128, TOT * 8], dt.int16, tag="eidx")
        poolw = cpool.tile([128, NB, 128], bf16, tag="poolw")
        zt4s = [cpool.tile([128, 4, ZC], fp8, tag=f"zt4_{i}", name=f"zt4_{i}")
                for i in range(3)]
        zpsb = cpool.tile([128, 387], f32, tag="zpsb")
        nc.vector.memset(zpsb[:], 0.0)

        for i in range(NB // 4):
            nc.sync.dma_start(xTs[i][:], xT0.ap()[:, i * 512:(i + 1) * 512])
        nc.sync.dma_start(waug1[:], waug1_d.ap())
        nc.sync.dma_start(waugn[:], waugn_d.ap())
        nc.sync.dma_start(wl3[:], wl3_d.ap().rearrange("k p m -> p k m"))
        nc.sync.dma_start(blp1[:], blp1_d.ap())
        nc.sync.dma_start(iota[:], iota_d.ap())
        nc.sync.dma_start(ident[:], ident_d.ap())
        nc.sync.dma_start(dst3[:], dst3_d.ap())
        nc.sync.dma_start(zidx[:], zidx_d.ap())
        nc.sync.dma_start(eidx[:], eidx_d.ap())
        nc.sync.dma_start(poolw[:], poolw_d.ap().rearrange("b p m -> p b m"))
        for zt4 in zt4s:
            # ones columns for the denominator; zero the padding tail once
            nc.vector.memset(zt4[:, :, 128:387:129], 1.0)
            nc.vector.memset(zt4[:, :, 387:512], 0.0)

        # defensively zero the consumed bytes of the er-pair tables so a
        # pair row read before its dense-phase write can never inject NaNs
        zf = cpool.tile([128, 2 * SHP // 128, 24], dt.uint8, tag="zf")
        nc.vector.memset(zf[:], 0)
        for et in erTabs:
            nc.sync.dma_start(
                et.ap().bitcast(dt.uint8)[:, 0:24].rearrange(
                    "(p a) c -> p a c", p=128), zf[:])

        psz_pool = tc.alloc_tile_pool(name="psz", bufs=cfg.psz_bufs,
                                      space="PSUM")
        g_pool = tc.alloc_tile_pool(name="g", bufs=cfg.g_bufs)
        r_pool = tc.alloc_tile_pool(name="r",
                                    bufs=cfg.g_bufs // cfg.er_w + 2)
        w_pool = tc.alloc_tile_pool(name="w", bufs=cfg.g_bufs)
        l_pool = tc.alloc_tile_pool(name="l", bufs=96)
        psb_pool = tc.alloc_tile_pool(name="psb", bufs=cfg.psb_bufs,
                                      space="PSUM")
        pst_pool = tc.alloc_tile_pool(name="pst", bufs=1, space="PSUM")
        psx_pool = tc.alloc_tile_pool(name="psx", bufs=1, space="PSUM")
        s_pool = tc.alloc_tile_pool(name="s", bufs=4)
        a_pool = tc.alloc_tile_pool(name="a", bufs=4)
        at_pool = tc.alloc_tile_pool(name="at", bufs=4)
        x_pool = tc.alloc_tile_pool(name="x", bufs=4)
        pp_pool = tc.alloc_tile_pool(name="pp", bufs=1, space="PSUM")

        ps_pool_acc = pp_pool.tile([128, 128], f32, tag="poolacc")

        def emit_dense_group(dlayer, tg):
            """Dense tiles 4tg..4tg+3 of layer `dlayer` -> fp8 z table shard."""
            par = dlayer % 2
            wa = waug1 if dlayer == 0 else waugn
            zt4 = zt4s[tg % 3]
            zt4b = zt4.bitcast(bf16)
            for j in range(4):
                t = tg * 4 + j
                psz = psz_pool.tile([128, 390], f32, tag="psz")
                nc.tensor.matmul(
                    psz[:], xTs[t // 4][:, (t % 4) * 128:(t % 4 + 1) * 128],
                    wa[:], start=True, stop=True)
                # GPSIMD cannot read PSUM on HW: one strided copy for the
                # three z segments (DVE/Act alternating) + per-node factors
                # Pl=e^el Ql=e^.2el Pr=e^er Qr=e^.2er (exp(lrelu(el+er)) ==
                # max(Pl*Pr, Ql*Qr)), so the edge phase needs no Act exp
                zs = zt4[:, j, 0:387].rearrange("p (s m) -> p s m", s=3, m=129)[
                    :, :, 0:128]
                zin = psz[:, 0:384].rearrange("p (s m) -> p s m", s=3, m=128)
                nc.scalar.copy(zs, zin)
                # [Pl|Ql|Pr|Qr] in two strided exps, fused by scale
                v = zt4b[:, j, 200:212].rearrange("p (q x) -> p q x", q=2)
                src2 = psz[:, 384:390].rearrange("p (q h) -> p q h", q=2)
                nc.scalar.activation(v[:, :, 0:3], src2, Act.Exp, scale=1.0)
                nc.scalar.activation(v[:, :, 3:6], src2, Act.Exp,
                                     scale=cfg.neg_slope)
            nc.sync.dma_start(
                zshs[par].ap()[tg * 512:(tg + 1) * 512, :]
                .rearrange("(c p) z -> p c z", c=4, p=128), zt4[:])
            if ((tg + 1) * 512) % PR == 0:
                do_zgather(nc, par, ((tg + 1) * 512) // PR - 1)
            # er-pair table rows for the previous 512-node group (whose +1
            # boundary row is now available); the very last group clamps its
            # final straddle row to itself (row SHP-1 is padding anyway).
            # half-layer batches keep the SP queue's DMACopy count low (each
            # dma_start holds the SP sequencer ~3us) while the first half is
            # still ready before the next layer's first er gathers need it
            def emit_ertab(d0, d1, last):
                et, zsh = erTabs[par], zshs[par]
                n = d1 - d0
                etv = et.ap()[2 * d0:2 * d1, :].rearrange(
                    "(d m) z -> d m z", m=2)
                src = zsh.ap()[d0:d1, 412:424]            # [Pr(d)|Qr(d)]
                nc.sync.dma_start(etv[:, 0, 0:12], src)
                nc.sync.dma_start(etv[:, 1, 0:12], src)
                nc.sync.dma_start(etv[:, 0, 12:24], src)
                n1 = n - 1 if last else n
                nc.sync.dma_start(etv[0:n1, 1, 12:24],
                                  zsh.ap()[d0 + 1:d0 + 1 + n1, 412:424])
                if last:
                    nc.sync.dma_start(etv[n - 1:n, 1, 12:24],
                                      zsh.ap()[SHP - 1:SHP, 412:424])
            half = (NB // 4) // 2
            if tg == half - 1:
                emit_ertab(0, 512 * half - 1, last=False)
            if tg == NB // 4 - 1:
                emit_ertab(512 * half - 1, SHP, last=True)

        # layer 0's dense phase runs standalone; layers 1-2 are emitted from
        # inside the previous layer's block ends (block b feeds dense tile b).
        for tg in range(NB // 4):
            emit_dense_group(0, tg)

        lh_uses = {}
        for layer in range(3):
            if not cfg.interleave and layer > 0:
                for tg in range(NB // 4):
                    emit_dense_group(layer, tg)
            zaug = zaugs[layer % 2]
            erTab = erTabs[layer % 2]
            # ---------------- edge phase (software pipelined) ---------------
            # unit = GC 256-edge chunks (2*GC z-gather groups, 1024 idxs);
            # er-pair gathers cover ERW units each (GC*ERW*128 idxs).
            ERW = cfg.er_w
            Gts, Rws, wts, psbs = {}, {}, {}, {}

            def emit_gathers(u):
                lo = u * GC
                gsz = min(GC, TOT - lo)
                Gt = g_pool.tile([128, 2 * GC, ZC], fp8, tag="G")
                nc.gpsimd.dma_gather(
                    Gt[:, 0:2 * gsz, :], zaug.ap(),
                    zidx[:, 16 * lo: 16 * (lo + gsz)],
                    num_idxs=gsz * 256, num_idxs_reg=gsz * 256,
                    elem_size=ZC, elem_step=ZC,
                    queue_num=(2 * u) % cfg.NQ)
                Gts[u] = (Gt, gsz)
                if u % ERW == 0:
                    c1 = min(lo + GC * ERW, TOT)
                    R = r_pool.tile([128, GC * ERW, ZC], fp8, tag="R")
                    nc.gpsimd.dma_gather(
                        R[:, 0:c1 - lo, :], erTab.ap(),
                        eidx[:, 8 * lo: 8 * c1],
                        num_idxs=(c1 - lo) * 128,
                        num_idxs_reg=(c1 - lo) * 128,
                        elem_size=ZC, elem_step=ZC,
                        queue_num=(2 * u + 1) % cfg.NQ)
                    Rws[u // ERW] = R

            def emit_wt(u):
                Gt, gsz = Gts[u]
                R = Rws[u // ERW]
                go = (u % ERW) * GC
                Gtb = Gt.bitcast(bf16)
                Rb = R.bitcast(bf16)
                # wt = max(Pl*Pr, Ql*Qr) == exp(leaky_relu(el+er)): pure DVE
                # ALU, no Act hop on the per-unit critical chain.  Pair rows
                # hold [Pr Qr] per slot as strided bf16 views.
                wt = w_pool.tile([128, 2 * GC, 3], f32, tag="wt")
                wtq = w_pool.tile([128, 2 * GC, 3], f32, tag="wtq")
                w4 = wt[:].rearrange("p (c s) h -> p c s h", s=2)[:, 0:gsz]
                q4 = wtq[:].rearrange("p (c s) h -> p c s h", s=2)[:, 0:gsz]
                rv = Rb[:, go:go + gsz, 0:12].rearrange(
                    "p c (s q h) -> p c s q h", s=2, q=2)
                gv = Gtb[:, 0:2 * gsz, 200:212].rearrange(
                    "p g (q h) -> p g q h", q=4)
                g4 = gv.rearrange("p (c s) q h -> p c s q h", s=2)
                eng = nc.gpsimd if cfg.wt_on_pool else nc.vector
                eng.tensor_tensor(
                    w4, g4[:, :, :, 0, :], rv[:, :, :, 0, :], Alu.mult)
                eng.tensor_tensor(
                    q4, g4[:, :, :, 1, :], rv[:, :, :, 1, :], Alu.mult)
                eng.tensor_tensor(
                    wt[:, 0:2 * gsz, :], wt[:, 0:2 * gsz, :],
                    wtq[:, 0:2 * gsz, :], Alu.max)
                wts[u] = wt

            def emit_compute(u):
                lo = u * GC
                hi = min(lo + GC, TOT)
                Gt, _ = Gts[u]
                wt = wts[u]
                for cc in range(lo, hi):
                    b = int(blk_of[cc])
                    if cc == cum[b]:
                        psbs[b] = psb_pool.tile([128, 387], f32, tag="psb",
                                                name="psb")
                        if cfg.prezero_psb:
                            nc.scalar.copy(psbs[b][:], zpsb[:])
                    psb = psbs[b]
                    c = cc - lo
                    for sl in (0, 1):
                        sub = 2 * cc + sl
                        S, W = int(S_cls[sub]), int(W_cls[sub])
                        tag = f"lh{S}_{W}"
                        for h in range(3):
                            lh = l_pool.tile([128, 128], bf16, tag=tag,
                                             bufs=cfg.lh_bufs, name="lh")
                            n_used = lh_uses[tag] = lh_uses.get(tag, 0) + 1
                            if W < 128 and n_used <= cfg.lh_bufs:
                                # fresh rotating buffer: zero the margins
                                # once; later builds rewrite exactly [S, S+W)
                                nc.vector.memset(lh[:], 0.0)
                            wsc = (dst3[:, sub:sub + 1] if cfg.ablate_wtdep
                                   else wt[:, 2 * c + sl, h:h + 1].opt())
                            nc.vector.tensor_scalar(
                                lh[:, S:S + W], iota[:, 0:W],
                                dst3[:, sub:sub + 1], wsc,
                                Alu.is_equal, Alu.mult)
                            nc.tensor.matmul(
                                psb[:, 129 * h:129 * h + 129], lh[:],
                                Gt[:, 2 * c + sl, 129 * h:129 * h + 129].opt(),
                                start=(not cfg.prezero_psb
                                       and sub == 2 * cum[b] and h == 0),
                                stop=(sub == 2 * cum[b + 1] - 1 and h == 2))

            def emit_block_end(b):
                psb = psbs.pop(b)
                r3 = s_pool.tile([128, 3], f32, tag="r3")
                # +eps: a denominator can only be 0 for padding rows (or a
                # lost race on the er tables); keep 1/den finite either way
                nc.vector.tensor_scalar_add(r3[:], psb[:, 128:387:129], 1e-12)
                nc.vector.reciprocal(r3[:], r3[:])
                agg = a_pool.tile([128, 384], bf16, tag="agg")
                for h in range(3):
                    nc.scalar.activation(
                        agg[:, 128 * h:128 * (h + 1)],
                        psb[:, 129 * h:129 * h + 128], Act.Copy,
                        scale=r3[:, h:h + 1])
                aggT = at_pool.tile([128, 3, 128], bf16, tag="aggT")
                pst = pst_pool.tile([128, 3, 128], bf16, tag="pst")
                for k in range(3):
                    nc.tensor.transpose(pst[:, k, :].opt(),
                                        agg[:, 128 * k:128 * (k + 1)],
                                        ident[:])
                nc.scalar.copy(aggT[:], pst[:])
                bw = min(128, SH - b * 128)
                if layer < 2:
                    psx = psx_pool.tile([128, 128], f32, tag="psx")
                    for k in range(3):
                        nc.tensor.matmul(psx[:], wl3[:, k, :].opt(),
                                         aggT[:, k, :].opt(),
                                         start=(k == 0), stop=(k == 2))
                    # bias-add and write x^T straight into the xT SBUF slice
                    # (Act engine; Identity accepts an AP bias unlike Copy)
                    xdst = xTs[b // 4][:, (b % 4) * 128:(b % 4) * 128 + bw]
                    nc.scalar.activation(xdst, psx[:, 0:bw], Act.Identity,
                                         bias=blp1[:])
                else:
                    psx = psx_pool.tile([128, 128], f32, tag="psx")
                    for k in range(3):
                        nc.tensor.matmul(psx[:], aggT[:, k, :].opt(),
                                         wl3[:, k, :].opt(),
                                         start=(k == 0), stop=(k == 2))
                    x3 = x_pool.tile([128, 128], bf16, tag="x3")
                    nc.scalar.copy(x3[:], psx[:])
                    nc.tensor.matmul(ps_pool_acc[:], poolw[:, b, :].opt(),
                                     x3[:], start=(b == 0), stop=(b == NB - 1))
                if cfg.interleave and layer < 2 and b % 4 == 3:
                    emit_dense_group(layer + 1, b // 4)

            LA = cfg.lookahead
            WL = cfg.wt_la
            for u0 in range(min(LA, n_units)):
                emit_gathers(u0)
            for u0 in range(min(WL, LA, n_units)):
                emit_wt(u0)
            for u in range(n_units):
                emit_compute(u)
                if u + LA < n_units:
                    emit_gathers(u + LA)
                # wt chains run several units ahead of compute so the Act
                # exp is never on the per-unit critical loop; their R
                # gathers landed LA-WL units earlier, so the DVE add never
                # head-of-line-blocks ready one-hot builds.
                if u + WL < n_units:
                    emit_wt(u + WL)
                if u >= 1:
                    for b in ends_in_unit.get(u - 1, []):
                        emit_block_end(b)
            for b in ends_in_unit.get(n_units - 1, []):
                emit_block_end(b)

            if layer == 0:
                nc.sync.dma_start(blp1[:], blpn_d.ap())

        po = x_pool.tile([128, 128], f32, tag="po")
        nc.vector.tensor_copy(po[:], ps_pool_acc[:])
        nc.sync.dma_start(pool_out.ap(), po[:])

        for p in (pp_pool, x_pool, at_pool, a_pool, s_pool,
                  psx_pool, pst_pool, psb_pool, l_pool, w_pool, r_pool,
                  g_pool, psz_pool, cpool):
            p.release()

    nc.compile()
    return nc


# ---------------------------------------------------------------------------
# top-level kernel
# ---------------------------------------------------------------------------

def _prepare(cfg, inputs):
    """Returns (nc_b, S_cls, W_cls, in_maps, host_meta)."""
    import ml_dtypes
    bf = ml_dtypes.bfloat16
    npf = np.asarray
    per_core_edges = []
    nc_b = np.zeros(cfg.NB, np.int64)
    for br, (s, d) in enumerate((("srcA", "dstA"), ("srcB", "dstB"))):
        src = npf(inputs[s]).astype(np.int64)
        dst = npf(inputs[d]).astype(np.int64)
        for q in range(cfg.gpb):
            es, ed, cnt, lo = _prep_edges(cfg, src, dst, q)
            per_core_edges.append((es, ed, lo))
            nc_b = np.maximum(nc_b, -(-cnt // 256))
    S_cls, W_cls = _chunk_windows(cfg, per_core_edges, nc_b)
    in_maps = []
    host_meta = {}
    iota = np.tile(np.arange(128, dtype=bf), (128, 1))
    ident = np.eye(128, dtype=bf)
    for br in range(2):
        sfx = "AB"[br]
        W1 = npf(inputs["W1" + sfx]); al1 = npf(inputs["al1" + sfx])
        ar1 = npf(inputs["ar1" + sfx]); b1 = npf(inputs["b1" + sfx])
        Wn = npf(inputs["Wn" + sfx]); aln = npf(inputs["aln" + sfx])
        arn = npf(inputs["arn" + sfx]); bn = npf(inputs["bn" + sfx])
        Wl = npf(inputs["Wl" + sfx]); bl = npf(inputs["bl" + sfx])
        gid = npf(inputs["gid" + sfx]).astype(np.int64)
        feats = npf(inputs["feats" + sfx]).astype(np.float32)
        waug1, waugn, wl3, blp1, blpn = _prep_branch_weights(
            cfg, W1, al1, ar1, b1, Wn, aln, arn, bn, Wl, bl)
        host_meta[sfx] = dict(blpn=blpn, gid=gid)
        for q in range(cfg.gpb):
            es, ed, lo = per_core_edges[br * cfg.gpb + q]
            zidx, eidx, dst3 = _pack_core(cfg, es, ed, lo, nc_b, S_cls)
            poolw = np.zeros((cfg.NB, 128, 128), bf)
            for b in range(cfg.NB):
                for i in range(min(128, cfg.SH - b * 128)):
                    n = lo + b * 128 + i
                    if n < cfg.N:
                        poolw[b, i, gid[n]] = 1.0
            xT0 = np.zeros((128, cfg.SHP), np.float32)
            xT0[:, :cfg.SH] = feats.T[:, lo:lo + cfg.SH]
            in_maps.append({
                "xT0": xT0.astype(bf), "waug1": waug1.astype(bf),
                "waugn": waugn.astype(bf),
                "wl3": wl3.astype(bf), "blp1": blp1.reshape(128, 1),
                "blpn": blpn.reshape(128, 1),
                "iota": iota, "ident": ident,
                "dst3": dst3, "zidx": zidx, "eidx": eidx, "poolw": poolw,
            })
    return nc_b, S_cls, W_cls, in_maps, host_meta


def _finalize(cfg, inputs, host_meta, pool_outs):
    """pool_outs: list of 8 [128,128] arrays -> full output [G,1] float64."""
    out = {}
    for br in range(2):
        sfx = "AB"[br]
        total = np.zeros((128, 128), np.float64)
        for q in range(cfg.gpb):
            total += pool_outs[br * cfg.gpb + q].astype(np.float64)
        gid = host_meta[sfx]["gid"]
        cnt = np.bincount(gid, minlength=128).astype(np.float64)
        total += cnt[:, None] * host_meta[sfx]["blpn"].astype(np.float64)[None, :]
        out[sfx] = (total / np.maximum(cnt[:, None], 1.0))[:cfg.G]
    cat = np.concatenate([out["A"], out["B"]], axis=1)
    Wo = np.asarray(inputs["Wo"]).astype(np.float64)
    bo = np.asarray(inputs["bo"]).astype(np.float64)
    return (cat @ Wo + bo).astype(np.float64)


_CACHE = {}


def kernel(**inputs):
    cfg = Cfg(N=inputs["featsA"].shape[0], G=128)
    nc_b, S_cls, W_cls, in_maps, host_meta = _prepare(cfg, inputs)
    key = ("prog", tuple(nc_b.tolist()), tuple(S_cls.tolist()),
           tuple(W_cls.tolist()))
    if key not in _CACHE:
        _CACHE[key] = build_program(cfg, nc_b, S_cls, W_cls)
    nc = _CACHE[key]
    from concourse.bass_utils import run_bass_kernel_spmd
    res = run_bass_kernel_spmd(nc, in_maps, list(range(cfg.n_cores)))
    pool_outs = [r["pool_out"] for r in res.results]
    return _finalize(cfg, inputs, host_meta, pool_outs)

